# revision 27
# baseline (speedup 1.0000x reference)
"""Trainium2 Bass kernel for nn_AudioDeviceModel (dilated causal conv stack).

Strategy (v3, chunk-pipelined polyphase):
  - Data parallel: batch 64 sharded as 8 rows per core across 8 cores.
  - Only the last FRAME=128 timesteps are output; receptive field 2047, so
    only the last 2174 input samples matter.  Per-layer output windows
    shrink accordingly (W_Y below).
  - Polyphase (even/odd time parity) layout: partitions =
    [parity(2) x batch(8) x channel(8)] = 128; per-parity half-widths HW_Y.
  - All matmul inputs bf16 (1 col/cycle PE streaming); psum fp32; the
    residual chain A_i is bf16 (measured 5.5e-3 rel err vs 2e-2 budget).
  - v3 scheduling changes vs v2:
    * Telescoping chain chunks: layer i's first drain chunk covers
      [0, q_i) with q_i = q_{i-1} - DIL[i], so the cross-layer serial
      chain (conv->relu->resid->stt->conv) advances through SINGLE
      chunk-sized hops per layer instead of ~full-width hops.  Chunk
      boundaries of consecutive layers line up exactly with the conv tap
      extent (q_i + 512 + DIL[i] == q_{i-1} + 512), so chunk K of layer i
      only waits on stt chunk K of layer i-1.
    * relu / resid / stt all run at chunk granularity on separate psum
      tiles, letting ACT (relu) and DVE (stt) of neighbouring chunks and
      layers overlap instead of ping-ponging serially.
    * Weight DMAs are issued in consumption order on the sync/gpsimd
      queues only: each dma_start costs the issuing engine's sequencer
      ~0.8us of descriptor writes, so ACT (relu) and DVE (stt) must not
      issue any.
    * PE warmup matmuls bridge from program start to the first weight
      arrival (no PE idle window), and a dummy ACT op at t=0 hoists the
      ~1.3us Relu ACT_TABLE_LOAD into the DMA window.
  - Layer 8 computes only the windows layer 9's dilated taps read
    ([0:320) and [512:576)); io biases folded into later conv biases on
    the host (kappa trick); mixer accumulated across layers into one
    psum tile; final bias-add writes the parity-interleaved [8,128].
"""

import sys

import numpy as np

try:
    import concourse.bass as bass
except ImportError:  # fresh environment without the site path
    sys.path.insert(0, "/opt/trn_rl_repo")
    import concourse.bass as bass

import ml_dtypes
import concourse.tile as tile
from concourse import bacc, mybir
from concourse.bass_utils import run_bass_kernel_spmd

N_LAYERS = 10
FRAME = 128
B, T = 64, 4096
N_CORES = 8
B_LOC = B // N_CORES  # 8 batch rows per core

DIL = [2**i for i in range(N_LAYERS)]
W_Y = [0] * N_LAYERS
W_H = [0] * N_LAYERS
W_Y[N_LAYERS - 1] = FRAME
for _i in range(N_LAYERS - 1, -1, -1):
    W_H[_i] = W_Y[_i] + 2 * DIL[_i]
    if _i > 0:
        W_Y[_i - 1] = W_H[_i]
W_X = W_H[0]  # 2174

# half-width (per parity) quantities
HW_Y = [w // 2 for w in W_Y]  # [1086,1084,1080,1072,1056,1024,960,832,576,64]
HW_H = [w // 2 for w in W_H]
XGW = HW_Y[0] + 2  # 1088, padded even

# telescoping chain-chunk boundary per layer (layers 0..7); layer i's
# first drain chunk [0, q_i) feeds layer i+1's first conv chunk exactly:
# q_{i+1} = q_i - DIL[i+1]  (DIL in half-cols == full dilation value)
Q = [384]
for _i in range(1, 8):
    Q.append(Q[-1] - DIL[_i])
# Q = [384, 382, 378, 370, 354, 322, 258, 130]

_F32 = mybir.dt.float32
_BF16 = mybir.dt.bfloat16
_ADD = mybir.AluOpType.add
NPBF16 = ml_dtypes.bfloat16


def _layer_chunks(i):
    """Drain/conv chunk boundaries for layer i (layers 0..7).

    [0, q) is the chain chunk; the rest is split into two EQUAL chunks
    (W-q == 702 for every layer, so c1 == c2 == 351 everywhere and the
    telescoping alignment conv-cK_i -> stt-cK_{i-1} holds exactly).  A
    512/190 split made the c1 sub-chain (conv+relu+resid+stt ~2.9us)
    exceed the ~2.5us layer slot, stalling the PE ~0.4us per layer;
    351/351 caps every sub-chain at ~2.3us.
    """
    w, q = HW_Y[i], Q[i]
    mid = q + (w - q + 1) // 2
    return [(0, q), (q, mid), (mid, w)]


def _build_program():
    nc = bacc.Bacc(
        "TRN2",
        target_bir_lowering=False,
        debug=False,
        enable_asserts=True,
        num_devices=N_CORES,
    )

    d_xg = nc.dram_tensor("xg", [32, XGW], _BF16, kind="ExternalInput").ap()
    d_w0r = nc.dram_tensor("w0r", [128, 384], _BF16, kind="ExternalInput").ap()
    d_wt = nc.dram_tensor("wt", [128, 27 * 128], _BF16, kind="ExternalInput").ap()
    d_wr = nc.dram_tensor("wr", [128, 8 * 128], _BF16, kind="ExternalInput").ap()
    d_wm = nc.dram_tensor("wm", [128, 400], _BF16, kind="ExternalInput").ap()
    d_cbmb = nc.dram_tensor("cbmb", [128, 11], _F32, kind="ExternalInput").ap()
    d_out = nc.dram_tensor("out", [B_LOC, FRAME], _F32, kind="ExternalOutput").ap()

    with tile.TileContext(nc) as tc:
        with (
            tc.tile_pool(name="wpool", bufs=1) as wpool,
            tc.tile_pool(name="apool", bufs=2) as apool,
            tc.tile_pool(name="ypool", bufs=2) as ypool,
            tc.tile_pool(name="opool", bufs=1) as opool,
            tc.tile_pool(name="pc", bufs=3, space="PSUM") as pcp,
            tc.tile_pool(name="pr", bufs=3, space="PSUM") as prp,
            tc.tile_pool(name="pm", bufs=1, space="PSUM") as pmp,
        ):
            XG = wpool.tile([32, XGW], _BF16, tag="XG", name="XG")
            W0R = wpool.tile([128, 384], _BF16, tag="W0R", name="W0R")
            WT = wpool.tile([128, 27 * 128], _BF16, tag="WT", name="WT")
            WR = wpool.tile([128, 8 * 128], _BF16, tag="WR", name="WR")
            WM = wpool.tile([128, 400], _BF16, tag="WM", name="WM")
            CBMB = wpool.tile([128, 11], _F32, tag="CBMB", name="CBMB")
            WUP = wpool.tile([128, 512], _BF16, tag="WUP", name="WUP")
            CB = CBMB[:, 0:10]

            # --- PE warmup: dummy matmuls on a zeroed tile keep the PE
            # busy (no idle window) from program start until the input
            # DMAs land, so HAM reaches K=8/8 (2.4GHz) early instead of
            # running the first half of the layers at 1.2GHz.
            nc.vector.memset(WUP[:, :], 0)
            # dummy ACT op at t=0: forces the walrus-inserted Relu
            # ACT_TABLE_LOAD (~1.3us) to run during the DMA window
            # instead of delaying the first real relu.
            DUMMY = opool.tile([8, 1], _F32, tag="dmy", name="dummy")
            nc.scalar.activation(DUMMY[:, :], WUP[0:8, 0:1],
                                 mybir.ActivationFunctionType.Relu)
            # dedicated psum bank for warmup/filler matmuls (never read,
            # never recycled — tag-private bufs=1 slot in the pc pool)
            pw = pcp.tile([128, 512], _F32, tag="pwarm", bufs=1,
                          padded_shape=[128, 512], name="pwarm")
            # Warmup bridges PE from program start to first weight
            # arrival.  (Longer warmups do NOT flip HAM earlier — the
            # unthrottle empirically trails the layer-pipeline start by
            # ~7us regardless — so anything past bridging is pure delay.)
            for k in range(5):
                nc.tensor.matmul(pw[:, 0:512], WUP[:, 0:128], WUP[:, 0:512],
                                 start=True, stop=True)

            def filler(n=256):
                """Dummy MM to plug a PE dependency stall so the HAM
                activity monitor keeps the PE clock at 2.4GHz."""
                nc.tensor.matmul(pw[:, 0:n], WUP[:, 0:128], WUP[:, 0:n],
                                 start=True, stop=True)

            # --- weight DMAs, in consumption order.  Only the sync and
            # gpsimd queues issue DMAs: a dma_start costs the issuing
            # engine's sequencer ~0.8us of descriptor writes, which must
            # not block ACT (relu) or DVE (stt).
            nc.sync.dma_start(XG[:, :], d_xg[:, :])
            nc.gpsimd.dma_start(W0R[:, :], d_w0r[:, :])
            nc.sync.dma_start(CBMB[:, :], d_cbmb[:, :])
            nc.gpsimd.dma_start(WR[:, 0:512], d_wr[:, 0:512])      # L1-4
            nc.sync.dma_start(WT[:, 0:1152], d_wt[:, 0:1152])      # L1-3
            nc.gpsimd.dma_start(WM[:, :], d_wm[:, :])
            nc.sync.dma_start(WT[:, 1152:2304], d_wt[:, 1152:2304])  # L4-6
            nc.gpsimd.dma_start(WR[:, 512:1024], d_wr[:, 512:1024])  # L5-8
            nc.sync.dma_start(WT[:, 2304:3456], d_wt[:, 2304:3456])  # L7-9

            A = [None] * (N_LAYERS + 1)
            Y = [None] * N_LAYERS

            pm = pmp.tile([40, 64], _F32, tag="pm", name="pm")

            def conv_mm(i, pc, o0, o1):
                """Conv taps for out cols [o0, o1) into psum tile pc."""
                if i == 0:
                    nc.tensor.matmul(
                        pc[:, 0:o1 - o0], W0R[0:32, 0:128], XG[0:32, o0:o1],
                        start=True, stop=True,
                    )
                    return
                hd = DIL[i] // 2
                c0 = (i - 1) * 3 * 128
                for t in range(3):
                    nc.tensor.matmul(
                        pc[:, 0:o1 - o0],
                        WT[:, c0 + t * 128:c0 + (t + 1) * 128],
                        A[i][:, o0 + t * hd:o1 + t * hd],
                        start=(t == 0), stop=(t == 2),
                    )

            def resid_mm(i, pr, o0, o1):
                """Residual 1x1 conv for out cols [o0,o1) into tile pr."""
                wri = W0R[:, 256:384] if i == 0 else WR[:, (i - 1) * 128:i * 128]
                nc.tensor.matmul(
                    pr[:, 0:o1 - o0], wri,
                    Y[i][:, o0:o1], start=True, stop=(i != 0),
                )
                if i == 0:
                    nc.tensor.matmul(
                        pr[:, 0:o1 - o0], W0R[0:32, 128:256],
                        XG[0:32, o0:o1], start=False, stop=True,
                    )

            def relu(i, pc, o0, o1):
                nc.scalar.activation(
                    Y[i][:, o0:o1], pc[:, 0:o1 - o0],
                    mybir.ActivationFunctionType.Relu, bias=CB[:, i:i + 1],
                )

            def stt(i, pr, s0, s1):
                """A_{i+1}[s0:s1] = bf16(resid + A_i shifted)."""
                if i == 0:
                    nc.vector.tensor_copy(A[1][:, s0:s1], pr[:, 0:s1 - s0])
                else:
                    nc.vector.scalar_tensor_tensor(
                        A[i + 1][:, s0:s1], pr[:, 0:s1 - s0], 0.0,
                        A[i][:, s0 + DIL[i]:s1 + DIL[i]], _ADD, _ADD,
                    )

            def emit_mixer(i):
                nc.tensor.matmul(
                    pm[0:40, 0:64], WM[:, i * 40:(i + 1) * 40],
                    Y[i][:, HW_Y[i] - 64:HW_Y[i]],
                    start=(i == 0), stop=(i == N_LAYERS - 1),
                    skip_group_check=True,
                )

            # --- pipelined layers 0..9 ---
            # Layers 0..7: chunks ck = [(0,q), (q,q+512), (q+512,W)] whose
            # boundaries telescope with the conv tap extent, so the chain
            # chunk (k=0) of layer i+1 starts as soon as stt_i chunk 0
            # lands.  Software pipeline: each layer's HEAD (chain chunk +
            # chunk-1 conv) is emitted before the previous layer's TAIL2
            # (chunk-2 resid/stt + mixer), keeping the PE FIFO free of
            # tail work when the next chain chunk becomes ready.
            CK = [_layer_chunks(i) for i in range(8)]
            CK.append([(0, 320), (512, 576)])  # layer 8 windows
            CK.append([(0, 64)])               # layer 9
            PCS = [None] * 10
            PRS = [None] * 10

            def alloc(i):
                w = 576 if i == 8 else HW_Y[i]
                Y[i] = ypool.tile([128, w], _BF16, tag="Y", name=f"Y{i}")
                if i < 9:
                    A[i + 1] = apool.tile([128, w], _BF16, tag="A",
                                          name=f"A{i+1}")
                PCS[i] = [
                    pcp.tile([128, b - a], _F32, tag="pc",
                             padded_shape=[128, 512], name=f"pc{i}_{k}")
                    for k, (a, b) in enumerate(CK[i])
                ]
                if i < 9:
                    PRS[i] = [
                        prp.tile([128, b - a], _F32, tag="pr",
                                 padded_shape=[128, 512], name=f"pr{i}_{k}")
                        for k, (a, b) in enumerate(CK[i])
                    ]

            def head_a(i):
                """Chain-chunk conv of layer i + chunk-1 conv + chunk-0
                relu.  Consumes only stt chunks 0/1 of layer i-1."""
                alloc(i)
                conv_mm(i, PCS[i][0], *CK[i][0])
                if i < 8:
                    conv_mm(i, PCS[i][1], *CK[i][1])
                relu(i, PCS[i][0], *CK[i][0])

            def head_c(i):
                resid_mm(i, PRS[i][0], *CK[i][0])
                stt(i, PRS[i][0], *CK[i][0])

            def head_b(i):
                """Last conv chunk of layer i (consumes stt chunk 2 of
                layer i-1, so must follow tail2(i-1)) + chunk-1 relu."""
                if i == 8:
                    conv_mm(8, PCS[8][1], *CK[8][1])
                    relu(8, PCS[8][1], *CK[8][1])
                else:
                    conv_mm(i, PCS[i][2], *CK[i][2])
                    relu(i, PCS[i][1], *CK[i][1])

            def tail(i):
                if i >= 8:
                    return
                resid_mm(i, PRS[i][1], *CK[i][1])
                stt(i, PRS[i][1], *CK[i][1])
                relu(i, PCS[i][2], *CK[i][2])

            def tail2(i):
                k = 1 if i == 8 else 2
                resid_mm(i, PRS[i][k], *CK[i][k])
                emit_mixer(i)
                stt(i, PRS[i][k], *CK[i][k])

            head_a(0)
            head_c(0)
            head_b(0)
            tail(0)
            for i in range(1, 9):
                if i >= 7:
                    # late layers: the serial L8->L9->output tail is gated
                    # by the c2-drain chain; prioritize it over the (now
                    # slack) chain chunk in every engine FIFO.
                    tail2(i - 1)
                    head_a(i)
                else:
                    head_a(i)
                    tail2(i - 1)
                head_c(i)
                head_b(i)
                tail(i)
            tail2(8)
            # layer 9 (consumes both stt windows of layer 8)
            alloc(9)
            conv_mm(9, PCS[9][0], 0, 64)
            relu(9, PCS[9][0], 0, 64)
            emit_mixer(9)

            # --- output: bias add + parity interleave + DMA ---
            out_sb = opool.tile([8, FRAME], _F32, tag="osb", name="osb")
            nc.scalar.activation(
                out_sb[0:8, 0:FRAME:2],
                pm[0:8, :],
                mybir.ActivationFunctionType.Identity,
                bias=CBMB[0:8, 10:11],
            )
            nc.vector.tensor_scalar_add(
                out_sb[0:8, 1:FRAME:2], pm[32:40, :], CBMB[32:40, 10:11]
            )
            nc.sync.dma_start(d_out[:, :], out_sb[:, :])

    nc.compile()
    return nc


def _host_weights(c0_kernel, c_kernels, c_biases, io_kernels, io_biases,
                  mixer_kernel, mixer_bias):
    """Block-diagonal bf16 weights + io-bias folding, shared by cores."""
    eye8 = np.eye(8, dtype=np.float32)
    eye16 = np.eye(16, dtype=np.float32)

    # layer-0 conv [32,128]: rows G0..G3 (4 parity-shifted x groups x 8
    # batch), cols [even out 64 | odd out 64]
    w0x = np.zeros((32, 256), dtype=np.float32)
    # even out: G0,G1,G2 get taps 0,1,2 ; odd out: G1,G2,G3 get taps 0,1,2
    for t in range(3):
        v = c0_kernel[t, 0, :][None, :]  # [1,8]
        w0x[t * 8:(t + 1) * 8, 0:64] = np.kron(eye8, v)
        w0x[(t + 1) * 8:(t + 2) * 8, 64:128] = np.kron(eye8, v)
    # x pass-through for resid0: G2 -> even, G3 -> odd, all channels 1
    ones = np.ones((1, 8), np.float32)
    w0x[16:24, 128:192] = np.kron(eye8, ones)
    w0x[24:32, 192:256] = np.kron(eye8, ones)
    # pack [w0x | kron(eye16, U_0)] into one early-DMA tensor
    w0r = np.zeros((128, 384), dtype=np.float32)
    w0r[0:32, 0:256] = w0x
    w0r[:, 256:384] = np.kron(eye16, io_kernels[0, 0])

    # conv taps layers 1..9: [128, 27*128], kron(eye16, W_t)
    wt = np.zeros((128, 27 * 128), dtype=np.float32)
    for i in range(9):
        for t in range(3):
            wt[:, ((i * 3) + t) * 128:((i * 3) + t + 1) * 128] = np.kron(
                eye16, c_kernels[i, t]
            )

    # resid layers 1..8: kron(eye16, U_i)
    wr = np.zeros((128, 8 * 128), dtype=np.float32)
    for i in range(1, 9):
        wr[:, (i - 1) * 128:i * 128] = np.kron(eye16, io_kernels[i, 0])

    # mixer: per layer [128,16]: both parities block-diag
    wm = np.zeros((128, 400), dtype=np.float32)
    for i in range(N_LAYERS):
        blk = np.kron(eye8, mixer_kernel[0, i * 8:(i + 1) * 8, 0][:, None])
        wm[0:64, i * 40:i * 40 + 8] = blk
        wm[64:128, i * 40 + 32:i * 40 + 40] = blk

    # conv biases with io biases folded through the conv taps
    cb = np.zeros((8, N_LAYERS), dtype=np.float64)
    kappa = np.zeros(8, dtype=np.float64)
    for i in range(N_LAYERS):
        if i == 0:
            adj = np.zeros(8)
        else:
            adj = np.einsum("kio,i->o", c_kernels[i - 1].astype(np.float64),
                            kappa)
        cb[:, i] = c_biases[i].astype(np.float64) + adj
        if i < N_LAYERS - 1:
            kappa = kappa + io_biases[i].astype(np.float64)
    cb = np.tile(cb.astype(np.float32), (16, 1))  # [128, 10]
    cbmb = np.zeros((128, 11), np.float32)
    cbmb[:, 0:10] = cb
    cbmb[0:40, 10] = float(np.asarray(mixer_bias).reshape(-1)[0])
    return dict(
        w0r=np.ascontiguousarray(w0r.astype(NPBF16)),
        wt=np.ascontiguousarray(wt.astype(NPBF16)),
        wr=np.ascontiguousarray(wr.astype(NPBF16)),
        wm=np.ascontiguousarray(wm.astype(NPBF16)),
        cbmb=cbmb,
    )


_NC_CACHE = None


def _get_nc():
    global _NC_CACHE
    if _NC_CACHE is None:
        _NC_CACHE = _build_program()
    return _NC_CACHE


def run(inputs, trace=False, **spmd_kwargs):
    """Run on 8 cores; returns (full_output [64,128], BassKernelResults)."""
    x = np.asarray(inputs["x"], dtype=np.float32)
    shared = _host_weights(
        np.asarray(inputs["c0_kernel"], np.float32),
        np.asarray(inputs["c_kernels"], np.float32),
        np.asarray(inputs["c_biases"], np.float32),
        np.asarray(inputs["io_kernels"], np.float32),
        np.asarray(inputs["io_biases"], np.float32),
        np.asarray(inputs["mixer_kernel"], np.float32),
        np.asarray(inputs["mixer_bias"], np.float32),
    )
    xw = x[:, T - W_X:]  # [64, 2174]
    in_maps = []
    for c in range(N_CORES):
        xc = xw[c * B_LOC:(c + 1) * B_LOC]  # [8, 2174]
        xg = np.zeros((32, XGW), dtype=np.float32)
        for g in range(4):
            # G_g[b, j] = x[b, 2j + g], j < HW_Y[0]
            sl = xc[:, g:g + 2 * HW_Y[0]:2]
            xg[g * 8:(g + 1) * 8, :sl.shape[1]] = sl
        m = dict(shared)
        m["xg"] = np.ascontiguousarray(xg.astype(NPBF16))
        in_maps.append(m)
    nc = _get_nc()
    res = run_bass_kernel_spmd(
        nc, in_maps, core_ids=list(range(N_CORES)), trace=trace, **spmd_kwargs
    )
    out = np.concatenate([res.results[c]["out"] for c in range(N_CORES)], axis=0)
    return out.astype(np.float32), res


def kernel(**inputs):
    out, _ = run(inputs, trace=False)
    return out


# revision 28
# speedup vs baseline: 1.0263x; 1.0263x over previous
"""Trainium2 Bass kernel for nn_AudioDeviceModel (dilated causal conv stack).

Strategy (v3, chunk-pipelined polyphase):
  - Data parallel: batch 64 sharded as 8 rows per core across 8 cores.
  - Only the last FRAME=128 timesteps are output; receptive field 2047, so
    only the last 2174 input samples matter.  Per-layer output windows
    shrink accordingly (W_Y below).
  - Polyphase (even/odd time parity) layout: partitions =
    [parity(2) x batch(8) x channel(8)] = 128; per-parity half-widths HW_Y.
  - All matmul inputs bf16 (1 col/cycle PE streaming); psum fp32; the
    residual chain A_i is bf16 (measured 5.5e-3 rel err vs 2e-2 budget).
  - v3 scheduling changes vs v2:
    * Telescoping chain chunks: layer i's first drain chunk covers
      [0, q_i) with q_i = q_{i-1} - DIL[i], so the cross-layer serial
      chain (conv->relu->resid->stt->conv) advances through SINGLE
      chunk-sized hops per layer instead of ~full-width hops.  Chunk
      boundaries of consecutive layers line up exactly with the conv tap
      extent (q_i + 512 + DIL[i] == q_{i-1} + 512), so chunk K of layer i
      only waits on stt chunk K of layer i-1.
    * relu / resid / stt all run at chunk granularity on separate psum
      tiles, letting ACT (relu) and DVE (stt) of neighbouring chunks and
      layers overlap instead of ping-ponging serially.
    * Weight DMAs are issued in consumption order on the sync/gpsimd
      queues only: each dma_start costs the issuing engine's sequencer
      ~0.8us of descriptor writes, so ACT (relu) and DVE (stt) must not
      issue any.
    * PE warmup matmuls bridge from program start to the first weight
      arrival (no PE idle window), and a dummy ACT op at t=0 hoists the
      ~1.3us Relu ACT_TABLE_LOAD into the DMA window.
  - Layer 8 computes only the windows layer 9's dilated taps read
    ([0:320) and [512:576)); io biases folded into later conv biases on
    the host (kappa trick); mixer accumulated across layers into one
    psum tile; final bias-add writes the parity-interleaved [8,128].
"""

import sys

import numpy as np

try:
    import concourse.bass as bass
except ImportError:  # fresh environment without the site path
    sys.path.insert(0, "/opt/trn_rl_repo")
    import concourse.bass as bass

import ml_dtypes
import concourse.tile as tile
from concourse import bacc, mybir
from concourse.bass_utils import run_bass_kernel_spmd

N_LAYERS = 10
FRAME = 128
B, T = 64, 4096
N_CORES = 8
B_LOC = B // N_CORES  # 8 batch rows per core

DIL = [2**i for i in range(N_LAYERS)]
W_Y = [0] * N_LAYERS
W_H = [0] * N_LAYERS
W_Y[N_LAYERS - 1] = FRAME
for _i in range(N_LAYERS - 1, -1, -1):
    W_H[_i] = W_Y[_i] + 2 * DIL[_i]
    if _i > 0:
        W_Y[_i - 1] = W_H[_i]
W_X = W_H[0]  # 2174

# half-width (per parity) quantities
HW_Y = [w // 2 for w in W_Y]  # [1086,1084,1080,1072,1056,1024,960,832,576,64]
HW_H = [w // 2 for w in W_H]
XGW = HW_Y[0] + 2  # 1088, padded even

# telescoping chain-chunk boundary per layer (layers 0..7); layer i's
# first drain chunk [0, q_i) feeds layer i+1's first conv chunk exactly:
# q_{i+1} = q_i - DIL[i+1]  (DIL in half-cols == full dilation value)
Q = [362]
for _i in range(1, 8):
    Q.append(Q[-1] - DIL[_i])
# Q = [362, 360, 356, 348, 332, 300, 236, 108]

_F32 = mybir.dt.float32
_BF16 = mybir.dt.bfloat16
_ADD = mybir.AluOpType.add
NPBF16 = ml_dtypes.bfloat16


def _layer_chunks(i):
    """Drain/conv chunk boundaries for layer i (layers 0..7).

    [0, q) is the chain chunk; the rest is split into two EQUAL chunks
    (W-q == 702 for every layer, so c1 == c2 == 351 everywhere and the
    telescoping alignment conv-cK_i -> stt-cK_{i-1} holds exactly).  A
    512/190 split made the c1 sub-chain (conv+relu+resid+stt ~2.9us)
    exceed the ~2.5us layer slot, stalling the PE ~0.4us per layer;
    351/351 caps every sub-chain at ~2.3us.
    """
    w, q = HW_Y[i], Q[i]
    mid = q + (w - q + 1) // 2
    return [(0, q), (q, mid), (mid, w)]


def _build_program():
    nc = bacc.Bacc(
        "TRN2",
        target_bir_lowering=False,
        debug=False,
        enable_asserts=True,
        num_devices=N_CORES,
    )

    d_xg = nc.dram_tensor("xg", [32, XGW], _BF16, kind="ExternalInput").ap()
    d_w0r = nc.dram_tensor("w0r", [128, 384], _BF16, kind="ExternalInput").ap()
    d_wt = nc.dram_tensor("wt", [128, 27 * 128], _BF16, kind="ExternalInput").ap()
    d_wr = nc.dram_tensor("wr", [128, 8 * 128], _BF16, kind="ExternalInput").ap()
    d_wm = nc.dram_tensor("wm", [128, 400], _BF16, kind="ExternalInput").ap()
    d_cbmb = nc.dram_tensor("cbmb", [128, 11], _F32, kind="ExternalInput").ap()
    d_out = nc.dram_tensor("out", [B_LOC, FRAME], _F32, kind="ExternalOutput").ap()

    with tile.TileContext(nc) as tc:
        with (
            tc.tile_pool(name="wpool", bufs=1) as wpool,
            tc.tile_pool(name="apool", bufs=2) as apool,
            tc.tile_pool(name="ypool", bufs=2) as ypool,
            tc.tile_pool(name="opool", bufs=1) as opool,
            tc.tile_pool(name="pc", bufs=3, space="PSUM") as pcp,
            tc.tile_pool(name="pr", bufs=3, space="PSUM") as prp,
            tc.tile_pool(name="pm", bufs=1, space="PSUM") as pmp,
        ):
            XG = wpool.tile([32, XGW], _BF16, tag="XG", name="XG")
            W0R = wpool.tile([128, 384], _BF16, tag="W0R", name="W0R")
            WT = wpool.tile([128, 27 * 128], _BF16, tag="WT", name="WT")
            WR = wpool.tile([128, 8 * 128], _BF16, tag="WR", name="WR")
            WM = wpool.tile([128, 400], _BF16, tag="WM", name="WM")
            CBMB = wpool.tile([128, 11], _F32, tag="CBMB", name="CBMB")
            WUP = wpool.tile([128, 512], _BF16, tag="WUP", name="WUP")
            CB = CBMB[:, 0:10]

            # --- PE warmup: dummy matmuls on a zeroed tile keep the PE
            # busy (no idle window) from program start until the input
            # DMAs land, so HAM reaches K=8/8 (2.4GHz) early instead of
            # running the first half of the layers at 1.2GHz.
            nc.vector.memset(WUP[:, :], 0)
            # dummy ACT op at t=0: forces the walrus-inserted Relu
            # ACT_TABLE_LOAD (~1.3us) to run during the DMA window
            # instead of delaying the first real relu.
            DUMMY = opool.tile([8, 1], _F32, tag="dmy", name="dummy")
            nc.scalar.activation(DUMMY[:, :], WUP[0:8, 0:1],
                                 mybir.ActivationFunctionType.Relu)
            # dedicated psum bank for warmup/filler matmuls (never read,
            # never recycled — tag-private bufs=1 slot in the pc pool)
            pw = pcp.tile([128, 512], _F32, tag="pwarm", bufs=1,
                          padded_shape=[128, 512], name="pwarm")
            # Warmup bridges PE from program start to first weight
            # arrival.  (Longer warmups do NOT flip HAM earlier — the
            # unthrottle empirically trails the layer-pipeline start by
            # ~7us regardless — so anything past bridging is pure delay.)
            for k in range(5):
                nc.tensor.matmul(pw[:, 0:512], WUP[:, 0:128], WUP[:, 0:512],
                                 start=True, stop=True)

            def filler(n=256):
                """Dummy MM to plug a PE dependency stall so the HAM
                activity monitor keeps the PE clock at 2.4GHz."""
                nc.tensor.matmul(pw[:, 0:n], WUP[:, 0:128], WUP[:, 0:n],
                                 start=True, stop=True)

            # --- weight DMAs, in consumption order.  Only the sync and
            # gpsimd queues issue DMAs: a dma_start costs the issuing
            # engine's sequencer ~0.8us of descriptor writes, which must
            # not block ACT (relu) or DVE (stt).
            nc.sync.dma_start(XG[:, :], d_xg[:, :])
            nc.gpsimd.dma_start(W0R[:, :], d_w0r[:, :])
            nc.sync.dma_start(CBMB[:, :], d_cbmb[:, :])
            nc.gpsimd.dma_start(WR[:, 0:512], d_wr[:, 0:512])      # L1-4
            nc.sync.dma_start(WT[:, 0:1152], d_wt[:, 0:1152])      # L1-3
            nc.gpsimd.dma_start(WM[:, :], d_wm[:, :])
            nc.sync.dma_start(WT[:, 1152:2304], d_wt[:, 1152:2304])  # L4-6
            nc.gpsimd.dma_start(WR[:, 512:1024], d_wr[:, 512:1024])  # L5-8
            nc.sync.dma_start(WT[:, 2304:3456], d_wt[:, 2304:3456])  # L7-9

            A = [None] * (N_LAYERS + 1)
            Y = [None] * N_LAYERS

            pm = pmp.tile([40, 64], _F32, tag="pm", name="pm")

            def conv_mm(i, pc, o0, o1):
                """Conv taps for out cols [o0, o1) into psum tile pc."""
                if i == 0:
                    nc.tensor.matmul(
                        pc[:, 0:o1 - o0], W0R[0:32, 0:128], XG[0:32, o0:o1],
                        start=True, stop=True,
                    )
                    return
                hd = DIL[i] // 2
                c0 = (i - 1) * 3 * 128
                for t in range(3):
                    nc.tensor.matmul(
                        pc[:, 0:o1 - o0],
                        WT[:, c0 + t * 128:c0 + (t + 1) * 128],
                        A[i][:, o0 + t * hd:o1 + t * hd],
                        start=(t == 0), stop=(t == 2),
                    )

            def resid_mm(i, pr, o0, o1):
                """Residual 1x1 conv for out cols [o0,o1) into tile pr."""
                wri = W0R[:, 256:384] if i == 0 else WR[:, (i - 1) * 128:i * 128]
                nc.tensor.matmul(
                    pr[:, 0:o1 - o0], wri,
                    Y[i][:, o0:o1], start=True, stop=(i != 0),
                )
                if i == 0:
                    nc.tensor.matmul(
                        pr[:, 0:o1 - o0], W0R[0:32, 128:256],
                        XG[0:32, o0:o1], start=False, stop=True,
                    )

            def relu(i, pc, o0, o1):
                nc.scalar.activation(
                    Y[i][:, o0:o1], pc[:, 0:o1 - o0],
                    mybir.ActivationFunctionType.Relu, bias=CB[:, i:i + 1],
                )

            def stt(i, pr, s0, s1):
                """A_{i+1}[s0:s1] = bf16(resid + A_i shifted)."""
                if i == 0:
                    nc.vector.tensor_copy(A[1][:, s0:s1], pr[:, 0:s1 - s0])
                else:
                    nc.vector.scalar_tensor_tensor(
                        A[i + 1][:, s0:s1], pr[:, 0:s1 - s0], 0.0,
                        A[i][:, s0 + DIL[i]:s1 + DIL[i]], _ADD, _ADD,
                    )

            def emit_mixer(i):
                nc.tensor.matmul(
                    pm[0:40, 0:64], WM[:, i * 40:(i + 1) * 40],
                    Y[i][:, HW_Y[i] - 64:HW_Y[i]],
                    start=(i == 0), stop=(i == N_LAYERS - 1),
                    skip_group_check=True,
                )

            # --- pipelined layers 0..9 ---
            # Layers 0..7: chunks ck = [(0,q), (q,q+512), (q+512,W)] whose
            # boundaries telescope with the conv tap extent, so the chain
            # chunk (k=0) of layer i+1 starts as soon as stt_i chunk 0
            # lands.  Software pipeline: each layer's HEAD (chain chunk +
            # chunk-1 conv) is emitted before the previous layer's TAIL2
            # (chunk-2 resid/stt + mixer), keeping the PE FIFO free of
            # tail work when the next chain chunk becomes ready.
            CK = [_layer_chunks(i) for i in range(8)]
            CK.append([(0, 320), (512, 576)])  # layer 8 windows
            CK.append([(0, 64)])               # layer 9
            PCS = [None] * 10
            PRS = [None] * 10

            def alloc(i):
                w = 576 if i == 8 else HW_Y[i]
                Y[i] = ypool.tile([128, w], _BF16, tag="Y", name=f"Y{i}")
                if i < 9:
                    A[i + 1] = apool.tile([128, w], _BF16, tag="A",
                                          name=f"A{i+1}")
                PCS[i] = [
                    pcp.tile([128, b - a], _F32, tag="pc",
                             padded_shape=[128, 512], name=f"pc{i}_{k}")
                    for k, (a, b) in enumerate(CK[i])
                ]
                if i < 9:
                    PRS[i] = [
                        prp.tile([128, b - a], _F32, tag="pr",
                                 padded_shape=[128, 512], name=f"pr{i}_{k}")
                        for k, (a, b) in enumerate(CK[i])
                    ]

            def head_a(i):
                """Chain-chunk conv of layer i + chunk-1 conv + chunk-0
                relu.  Consumes only stt chunks 0/1 of layer i-1."""
                alloc(i)
                conv_mm(i, PCS[i][0], *CK[i][0])
                if i < 8:
                    conv_mm(i, PCS[i][1], *CK[i][1])
                relu(i, PCS[i][0], *CK[i][0])

            def head_c(i):
                resid_mm(i, PRS[i][0], *CK[i][0])
                stt(i, PRS[i][0], *CK[i][0])

            def head_b(i):
                """Last conv chunk of layer i (consumes stt chunk 2 of
                layer i-1, so must follow tail2(i-1)) + chunk-1 relu."""
                if i == 8:
                    conv_mm(8, PCS[8][1], *CK[8][1])
                    relu(8, PCS[8][1], *CK[8][1])
                else:
                    conv_mm(i, PCS[i][2], *CK[i][2])
                    relu(i, PCS[i][1], *CK[i][1])

            def tail(i):
                if i >= 8:
                    return
                resid_mm(i, PRS[i][1], *CK[i][1])
                stt(i, PRS[i][1], *CK[i][1])
                relu(i, PCS[i][2], *CK[i][2])

            def tail2(i):
                k = 1 if i == 8 else 2
                resid_mm(i, PRS[i][k], *CK[i][k])
                emit_mixer(i)
                stt(i, PRS[i][k], *CK[i][k])

            head_a(0)
            head_c(0)
            head_b(0)
            tail(0)
            for i in range(1, 9):
                if i >= 7:
                    # late layers: the serial L8->L9->output tail is gated
                    # by the c2-drain chain; prioritize it over the (now
                    # slack) chain chunk in every engine FIFO.
                    tail2(i - 1)
                    head_a(i)
                else:
                    head_a(i)
                    tail2(i - 1)
                head_c(i)
                head_b(i)
                tail(i)
            tail2(8)
            # layer 9 (consumes both stt windows of layer 8)
            alloc(9)
            conv_mm(9, PCS[9][0], 0, 64)
            relu(9, PCS[9][0], 0, 64)
            emit_mixer(9)

            # --- output: bias add + parity interleave + DMA ---
            out_sb = opool.tile([8, FRAME], _F32, tag="osb", name="osb")
            nc.scalar.activation(
                out_sb[0:8, 0:FRAME:2],
                pm[0:8, :],
                mybir.ActivationFunctionType.Identity,
                bias=CBMB[0:8, 10:11],
            )
            nc.vector.tensor_scalar_add(
                out_sb[0:8, 1:FRAME:2], pm[32:40, :], CBMB[32:40, 10:11]
            )
            nc.sync.dma_start(d_out[:, :], out_sb[:, :])

    nc.compile()
    return nc


def _host_weights(c0_kernel, c_kernels, c_biases, io_kernels, io_biases,
                  mixer_kernel, mixer_bias):
    """Block-diagonal bf16 weights + io-bias folding, shared by cores."""
    eye8 = np.eye(8, dtype=np.float32)
    eye16 = np.eye(16, dtype=np.float32)

    # layer-0 conv [32,128]: rows G0..G3 (4 parity-shifted x groups x 8
    # batch), cols [even out 64 | odd out 64]
    w0x = np.zeros((32, 256), dtype=np.float32)
    # even out: G0,G1,G2 get taps 0,1,2 ; odd out: G1,G2,G3 get taps 0,1,2
    for t in range(3):
        v = c0_kernel[t, 0, :][None, :]  # [1,8]
        w0x[t * 8:(t + 1) * 8, 0:64] = np.kron(eye8, v)
        w0x[(t + 1) * 8:(t + 2) * 8, 64:128] = np.kron(eye8, v)
    # x pass-through for resid0: G2 -> even, G3 -> odd, all channels 1
    ones = np.ones((1, 8), np.float32)
    w0x[16:24, 128:192] = np.kron(eye8, ones)
    w0x[24:32, 192:256] = np.kron(eye8, ones)
    # pack [w0x | kron(eye16, U_0)] into one early-DMA tensor
    w0r = np.zeros((128, 384), dtype=np.float32)
    w0r[0:32, 0:256] = w0x
    w0r[:, 256:384] = np.kron(eye16, io_kernels[0, 0])

    # conv taps layers 1..9: [128, 27*128], kron(eye16, W_t)
    wt = np.zeros((128, 27 * 128), dtype=np.float32)
    for i in range(9):
        for t in range(3):
            wt[:, ((i * 3) + t) * 128:((i * 3) + t + 1) * 128] = np.kron(
                eye16, c_kernels[i, t]
            )

    # resid layers 1..8: kron(eye16, U_i)
    wr = np.zeros((128, 8 * 128), dtype=np.float32)
    for i in range(1, 9):
        wr[:, (i - 1) * 128:i * 128] = np.kron(eye16, io_kernels[i, 0])

    # mixer: per layer [128,16]: both parities block-diag
    wm = np.zeros((128, 400), dtype=np.float32)
    for i in range(N_LAYERS):
        blk = np.kron(eye8, mixer_kernel[0, i * 8:(i + 1) * 8, 0][:, None])
        wm[0:64, i * 40:i * 40 + 8] = blk
        wm[64:128, i * 40 + 32:i * 40 + 40] = blk

    # conv biases with io biases folded through the conv taps
    cb = np.zeros((8, N_LAYERS), dtype=np.float64)
    kappa = np.zeros(8, dtype=np.float64)
    for i in range(N_LAYERS):
        if i == 0:
            adj = np.zeros(8)
        else:
            adj = np.einsum("kio,i->o", c_kernels[i - 1].astype(np.float64),
                            kappa)
        cb[:, i] = c_biases[i].astype(np.float64) + adj
        if i < N_LAYERS - 1:
            kappa = kappa + io_biases[i].astype(np.float64)
    cb = np.tile(cb.astype(np.float32), (16, 1))  # [128, 10]
    cbmb = np.zeros((128, 11), np.float32)
    cbmb[:, 0:10] = cb
    cbmb[0:40, 10] = float(np.asarray(mixer_bias).reshape(-1)[0])
    return dict(
        w0r=np.ascontiguousarray(w0r.astype(NPBF16)),
        wt=np.ascontiguousarray(wt.astype(NPBF16)),
        wr=np.ascontiguousarray(wr.astype(NPBF16)),
        wm=np.ascontiguousarray(wm.astype(NPBF16)),
        cbmb=cbmb,
    )


_NC_CACHE = None


def _get_nc():
    global _NC_CACHE
    if _NC_CACHE is None:
        _NC_CACHE = _build_program()
    return _NC_CACHE


def run(inputs, trace=False, **spmd_kwargs):
    """Run on 8 cores; returns (full_output [64,128], BassKernelResults)."""
    x = np.asarray(inputs["x"], dtype=np.float32)
    shared = _host_weights(
        np.asarray(inputs["c0_kernel"], np.float32),
        np.asarray(inputs["c_kernels"], np.float32),
        np.asarray(inputs["c_biases"], np.float32),
        np.asarray(inputs["io_kernels"], np.float32),
        np.asarray(inputs["io_biases"], np.float32),
        np.asarray(inputs["mixer_kernel"], np.float32),
        np.asarray(inputs["mixer_bias"], np.float32),
    )
    xw = x[:, T - W_X:]  # [64, 2174]
    in_maps = []
    for c in range(N_CORES):
        xc = xw[c * B_LOC:(c + 1) * B_LOC]  # [8, 2174]
        xg = np.zeros((32, XGW), dtype=np.float32)
        for g in range(4):
            # G_g[b, j] = x[b, 2j + g], j < HW_Y[0]
            sl = xc[:, g:g + 2 * HW_Y[0]:2]
            xg[g * 8:(g + 1) * 8, :sl.shape[1]] = sl
        m = dict(shared)
        m["xg"] = np.ascontiguousarray(xg.astype(NPBF16))
        in_maps.append(m)
    nc = _get_nc()
    res = run_bass_kernel_spmd(
        nc, in_maps, core_ids=list(range(N_CORES)), trace=trace, **spmd_kwargs
    )
    out = np.concatenate([res.results[c]["out"] for c in range(N_CORES)], axis=0)
    return out.astype(np.float32), res


def kernel(**inputs):
    out, _ = run(inputs, trace=False)
    return out


# revision 30
# speedup vs baseline: 1.0489x; 1.0220x over previous
"""Trainium2 Bass kernel for nn_AudioDeviceModel (dilated causal conv stack).

Strategy (v3, chunk-pipelined polyphase):
  - Data parallel: batch 64 sharded as 8 rows per core across 8 cores.
  - Only the last FRAME=128 timesteps are output; receptive field 2047, so
    only the last 2174 input samples matter.  Per-layer output windows
    shrink accordingly (W_Y below).
  - Polyphase (even/odd time parity) layout: partitions =
    [parity(2) x batch(8) x channel(8)] = 128; per-parity half-widths HW_Y.
  - All matmul inputs bf16 (1 col/cycle PE streaming); psum fp32; the
    residual chain A_i is bf16 (measured 5.5e-3 rel err vs 2e-2 budget).
  - v3 scheduling changes vs v2:
    * Telescoping chain chunks: layer i's first drain chunk covers
      [0, q_i) with q_i = q_{i-1} - DIL[i], so the cross-layer serial
      chain (conv->relu->resid->stt->conv) advances through SINGLE
      chunk-sized hops per layer instead of ~full-width hops.  Chunk
      boundaries of consecutive layers line up exactly with the conv tap
      extent (q_i + 512 + DIL[i] == q_{i-1} + 512), so chunk K of layer i
      only waits on stt chunk K of layer i-1.
    * relu / resid / stt all run at chunk granularity on separate psum
      tiles, letting ACT (relu) and DVE (stt) of neighbouring chunks and
      layers overlap instead of ping-ponging serially.
    * Weight DMAs are issued in consumption order on the sync/gpsimd
      queues only: each dma_start costs the issuing engine's sequencer
      ~0.8us of descriptor writes, so ACT (relu) and DVE (stt) must not
      issue any.
    * PE warmup matmuls bridge from program start to the first weight
      arrival (no PE idle window), and a dummy ACT op at t=0 hoists the
      ~1.3us Relu ACT_TABLE_LOAD into the DMA window.
  - Layer 8 computes only the windows layer 9's dilated taps read
    ([0:320) and [512:576)); io biases folded into later conv biases on
    the host (kappa trick); mixer accumulated across layers into one
    psum tile; final bias-add writes the parity-interleaved [8,128].
"""

import sys

import numpy as np

try:
    import concourse.bass as bass
except ImportError:  # fresh environment without the site path
    sys.path.insert(0, "/opt/trn_rl_repo")
    import concourse.bass as bass

import ml_dtypes
import concourse.tile as tile
from concourse import bacc, mybir
from concourse.bass_utils import run_bass_kernel_spmd

N_LAYERS = 10
FRAME = 128
B, T = 64, 4096
N_CORES = 8
B_LOC = B // N_CORES  # 8 batch rows per core

DIL = [2**i for i in range(N_LAYERS)]
W_Y = [0] * N_LAYERS
W_H = [0] * N_LAYERS
W_Y[N_LAYERS - 1] = FRAME
for _i in range(N_LAYERS - 1, -1, -1):
    W_H[_i] = W_Y[_i] + 2 * DIL[_i]
    if _i > 0:
        W_Y[_i - 1] = W_H[_i]
W_X = W_H[0]  # 2174

# half-width (per parity) quantities
HW_Y = [w // 2 for w in W_Y]  # [1086,1084,1080,1072,1056,1024,960,832,576,64]
HW_H = [w // 2 for w in W_H]
XGW = HW_Y[0] + 2  # 1088, padded even

# telescoping chain-chunk boundary per layer (layers 0..7); layer i's
# first drain chunk [0, q_i) feeds layer i+1's first conv chunk exactly:
# q_{i+1} = q_i - DIL[i+1]  (DIL in half-cols == full dilation value)
Q = [362]
for _i in range(1, 8):
    Q.append(Q[-1] - DIL[_i])
# Q = [362, 360, 356, 348, 332, 300, 236, 108]

_F32 = mybir.dt.float32
_BF16 = mybir.dt.bfloat16
_ADD = mybir.AluOpType.add
NPBF16 = ml_dtypes.bfloat16


def _layer_chunks(i):
    """Drain/conv chunk boundaries for layer i (layers 0..7).

    [0, q) is the chain chunk; the rest is split into two EQUAL chunks
    (W-q == 702 for every layer, so c1 == c2 == 351 everywhere and the
    telescoping alignment conv-cK_i -> stt-cK_{i-1} holds exactly).  A
    512/190 split made the c1 sub-chain (conv+relu+resid+stt ~2.9us)
    exceed the ~2.5us layer slot, stalling the PE ~0.4us per layer;
    351/351 caps every sub-chain at ~2.3us.
    """
    w, q = HW_Y[i], Q[i]
    mid = q + (w - q + 1) // 2
    return [(0, q), (q, mid), (mid, w)]


def _build_program():
    nc = bacc.Bacc(
        "TRN2",
        target_bir_lowering=False,
        debug=False,
        enable_asserts=True,
        num_devices=N_CORES,
    )

    d_xg = nc.dram_tensor("xg", [32, XGW], _BF16, kind="ExternalInput").ap()
    d_xb = nc.dram_tensor("xb", [128, XGW], _BF16, kind="ExternalInput").ap()
    d_w0r = nc.dram_tensor("w0r", [128, 384], _BF16, kind="ExternalInput").ap()
    d_wt = nc.dram_tensor("wt", [128, 27 * 128], _BF16, kind="ExternalInput").ap()
    d_wr = nc.dram_tensor("wr", [128, 8 * 128], _BF16, kind="ExternalInput").ap()
    d_wm = nc.dram_tensor("wm", [128, 400], _BF16, kind="ExternalInput").ap()
    d_cbmb = nc.dram_tensor("cbmb", [128, 11], _F32, kind="ExternalInput").ap()
    d_out = nc.dram_tensor("out", [B_LOC, FRAME], _F32, kind="ExternalOutput").ap()

    with tile.TileContext(nc) as tc:
        with (
            tc.tile_pool(name="wpool", bufs=1) as wpool,
            tc.tile_pool(name="apool", bufs=2) as apool,
            tc.tile_pool(name="ypool", bufs=2) as ypool,
            tc.tile_pool(name="opool", bufs=1) as opool,
            tc.tile_pool(name="pc", bufs=3, space="PSUM") as pcp,
            tc.tile_pool(name="pr", bufs=3, space="PSUM") as prp,
            tc.tile_pool(name="pm", bufs=1, space="PSUM") as pmp,
        ):
            XG = wpool.tile([32, XGW], _BF16, tag="XG", name="XG")
            XB = wpool.tile([128, XGW], _BF16, tag="XB", name="XB")
            W0R = wpool.tile([128, 384], _BF16, tag="W0R", name="W0R")
            WT = wpool.tile([128, 27 * 128], _BF16, tag="WT", name="WT")
            WR = wpool.tile([128, 8 * 128], _BF16, tag="WR", name="WR")
            WM = wpool.tile([128, 400], _BF16, tag="WM", name="WM")
            CBMB = wpool.tile([128, 11], _F32, tag="CBMB", name="CBMB")
            WUP = wpool.tile([128, 512], _BF16, tag="WUP", name="WUP")
            CB = CBMB[:, 0:10]

            # --- PE warmup: dummy matmuls on a zeroed tile keep the PE
            # busy (no idle window) from program start until the input
            # DMAs land, so HAM reaches K=8/8 (2.4GHz) early instead of
            # running the first half of the layers at 1.2GHz.
            nc.vector.memset(WUP[:, :], 0)
            # dummy ACT op at t=0: forces the walrus-inserted Relu
            # ACT_TABLE_LOAD (~1.3us) to run during the DMA window
            # instead of delaying the first real relu.
            DUMMY = opool.tile([8, 1], _F32, tag="dmy", name="dummy")
            nc.scalar.activation(DUMMY[:, :], WUP[0:8, 0:1],
                                 mybir.ActivationFunctionType.Relu)
            # dedicated psum bank for warmup/filler matmuls (never read,
            # never recycled — tag-private bufs=1 slot in the pc pool)
            pw = pcp.tile([128, 512], _F32, tag="pwarm", bufs=1,
                          padded_shape=[128, 512], name="pwarm")
            # Warmup bridges PE from program start to first weight
            # arrival.  (Longer warmups do NOT flip HAM earlier — the
            # unthrottle empirically trails the layer-pipeline start by
            # ~7us regardless — so anything past bridging is pure delay.)
            for k in range(5):
                nc.tensor.matmul(pw[:, 0:512], WUP[:, 0:128], WUP[:, 0:512],
                                 start=True, stop=True)

            def filler(n=256):
                """Dummy MM to plug a PE dependency stall so the HAM
                activity monitor keeps the PE clock at 2.4GHz."""
                nc.tensor.matmul(pw[:, 0:n], WUP[:, 0:128], WUP[:, 0:n],
                                 start=True, stop=True)

            # --- weight DMAs, in consumption order.  Only the sync and
            # gpsimd queues issue DMAs: a dma_start costs the issuing
            # engine's sequencer ~0.8us of descriptor writes, which must
            # not block ACT (relu) or DVE (stt).
            nc.sync.dma_start(XG[:, :], d_xg[:, :])
            nc.gpsimd.dma_start(W0R[:, :], d_w0r[:, :])
            nc.gpsimd.dma_start(XB[:, :], d_xb[:, :])
            nc.sync.dma_start(CBMB[:, :], d_cbmb[:, :])
            nc.gpsimd.dma_start(WR[:, 0:512], d_wr[:, 0:512])      # L1-4
            nc.sync.dma_start(WT[:, 0:1152], d_wt[:, 0:1152])      # L1-3
            nc.gpsimd.dma_start(WM[:, :], d_wm[:, :])
            nc.sync.dma_start(WT[:, 1152:2304], d_wt[:, 1152:2304])  # L4-6
            nc.gpsimd.dma_start(WR[:, 512:1024], d_wr[:, 512:1024])  # L5-8
            nc.sync.dma_start(WT[:, 2304:3456], d_wt[:, 2304:3456])  # L7-9

            A = [None] * (N_LAYERS + 1)
            Y = [None] * N_LAYERS

            pm = pmp.tile([40, 64], _F32, tag="pm", name="pm")

            def conv_mm(i, pc, o0, o1):
                """Conv taps for out cols [o0, o1) into psum tile pc."""
                if i == 0:
                    nc.tensor.matmul(
                        pc[:, 0:o1 - o0], W0R[0:32, 0:128], XG[0:32, o0:o1],
                        start=True, stop=True,
                    )
                    return
                hd = DIL[i] // 2
                c0 = (i - 1) * 3 * 128
                for t in range(3):
                    nc.tensor.matmul(
                        pc[:, 0:o1 - o0],
                        WT[:, c0 + t * 128:c0 + (t + 1) * 128],
                        A[i][:, o0 + t * hd:o1 + t * hd],
                        start=(t == 0), stop=(t == 2),
                    )

            def resid_mm(i, pr, o0, o1):
                """Residual 1x1 conv for out cols [o0,o1) into tile pr."""
                wri = W0R[:, 256:384] if i == 0 else WR[:, (i - 1) * 128:i * 128]
                nc.tensor.matmul(
                    pr[:, 0:o1 - o0], wri,
                    Y[i][:, o0:o1], start=True, stop=True,
                )

            def relu(i, pc, o0, o1):
                nc.scalar.activation(
                    Y[i][:, o0:o1], pc[:, 0:o1 - o0],
                    mybir.ActivationFunctionType.Relu, bias=CB[:, i:i + 1],
                )

            def stt(i, pr, s0, s1):
                """A_{i+1}[s0:s1] = bf16(resid + A_i shifted)."""
                if i == 0:
                    nc.vector.scalar_tensor_tensor(
                        A[1][:, s0:s1], pr[:, 0:s1 - s0], 0.0,
                        XB[:, s0:s1], _ADD, _ADD,
                    )
                else:
                    nc.vector.scalar_tensor_tensor(
                        A[i + 1][:, s0:s1], pr[:, 0:s1 - s0], 0.0,
                        A[i][:, s0 + DIL[i]:s1 + DIL[i]], _ADD, _ADD,
                    )

            def emit_mixer(i):
                nc.tensor.matmul(
                    pm[0:40, 0:64], WM[:, i * 40:(i + 1) * 40],
                    Y[i][:, HW_Y[i] - 64:HW_Y[i]],
                    start=(i == 0), stop=(i == N_LAYERS - 1),
                    skip_group_check=True,
                )

            # --- pipelined layers 0..9 ---
            # Layers 0..7: chunks ck = [(0,q), (q,q+512), (q+512,W)] whose
            # boundaries telescope with the conv tap extent, so the chain
            # chunk (k=0) of layer i+1 starts as soon as stt_i chunk 0
            # lands.  Software pipeline: each layer's HEAD (chain chunk +
            # chunk-1 conv) is emitted before the previous layer's TAIL2
            # (chunk-2 resid/stt + mixer), keeping the PE FIFO free of
            # tail work when the next chain chunk becomes ready.
            CK = [_layer_chunks(i) for i in range(8)]
            CK.append([(0, 320), (512, 576)])  # layer 8 windows
            CK.append([(0, 64)])               # layer 9
            PCS = [None] * 10
            PRS = [None] * 10

            def alloc(i):
                w = 576 if i == 8 else HW_Y[i]
                Y[i] = ypool.tile([128, w], _BF16, tag="Y", name=f"Y{i}")
                if i < 9:
                    A[i + 1] = apool.tile([128, w], _BF16, tag="A",
                                          name=f"A{i+1}")
                PCS[i] = [
                    pcp.tile([128, b - a], _F32, tag="pc",
                             padded_shape=[128, 512], name=f"pc{i}_{k}")
                    for k, (a, b) in enumerate(CK[i])
                ]
                if i < 9:
                    PRS[i] = [
                        prp.tile([128, b - a], _F32, tag="pr",
                                 padded_shape=[128, 512], name=f"pr{i}_{k}")
                        for k, (a, b) in enumerate(CK[i])
                    ]

            def head_a(i):
                """Chain-chunk conv of layer i + chunk-1 conv + chunk-0
                relu.  Consumes only stt chunks 0/1 of layer i-1."""
                alloc(i)
                conv_mm(i, PCS[i][0], *CK[i][0])
                if i < 8:
                    conv_mm(i, PCS[i][1], *CK[i][1])
                relu(i, PCS[i][0], *CK[i][0])

            def head_c(i):
                resid_mm(i, PRS[i][0], *CK[i][0])
                stt(i, PRS[i][0], *CK[i][0])

            def head_b(i):
                """Last conv chunk of layer i (consumes stt chunk 2 of
                layer i-1, so must follow tail2(i-1)) + chunk-1 relu."""
                if i == 8:
                    conv_mm(8, PCS[8][1], *CK[8][1])
                    relu(8, PCS[8][1], *CK[8][1])
                else:
                    conv_mm(i, PCS[i][2], *CK[i][2])
                    relu(i, PCS[i][1], *CK[i][1])

            def tail(i):
                if i >= 8:
                    return
                resid_mm(i, PRS[i][1], *CK[i][1])
                stt(i, PRS[i][1], *CK[i][1])
                relu(i, PCS[i][2], *CK[i][2])

            def tail2(i):
                k = 1 if i == 8 else 2
                resid_mm(i, PRS[i][k], *CK[i][k])
                emit_mixer(i)
                stt(i, PRS[i][k], *CK[i][k])

            head_a(0)
            head_c(0)
            head_b(0)
            tail(0)
            for i in range(1, 9):
                if i >= 7:
                    # late layers: the serial L8->L9->output tail is gated
                    # by the c2-drain chain; prioritize it over the (now
                    # slack) chain chunk in every engine FIFO.
                    tail2(i - 1)
                    head_a(i)
                else:
                    head_a(i)
                    tail2(i - 1)
                head_c(i)
                head_b(i)
                tail(i)
            tail2(8)
            # layer 9 (consumes both stt windows of layer 8)
            alloc(9)
            conv_mm(9, PCS[9][0], 0, 64)
            relu(9, PCS[9][0], 0, 64)
            emit_mixer(9)

            # --- output: bias add + parity interleave + DMA ---
            out_sb = opool.tile([8, FRAME], _F32, tag="osb", name="osb")
            nc.scalar.activation(
                out_sb[0:8, 0:FRAME:2],
                pm[0:8, :],
                mybir.ActivationFunctionType.Identity,
                bias=CBMB[0:8, 10:11],
            )
            nc.vector.tensor_scalar_add(
                out_sb[0:8, 1:FRAME:2], pm[32:40, :], CBMB[32:40, 10:11]
            )
            nc.sync.dma_start(d_out[:, :], out_sb[:, :])

    nc.compile()
    return nc


def _host_weights(c0_kernel, c_kernels, c_biases, io_kernels, io_biases,
                  mixer_kernel, mixer_bias):
    """Block-diagonal bf16 weights + io-bias folding, shared by cores."""
    eye8 = np.eye(8, dtype=np.float32)
    eye16 = np.eye(16, dtype=np.float32)

    # layer-0 conv [32,128]: rows G0..G3 (4 parity-shifted x groups x 8
    # batch), cols [even out 64 | odd out 64]
    w0x = np.zeros((32, 256), dtype=np.float32)
    # even out: G0,G1,G2 get taps 0,1,2 ; odd out: G1,G2,G3 get taps 0,1,2
    for t in range(3):
        v = c0_kernel[t, 0, :][None, :]  # [1,8]
        w0x[t * 8:(t + 1) * 8, 0:64] = np.kron(eye8, v)
        w0x[(t + 1) * 8:(t + 2) * 8, 64:128] = np.kron(eye8, v)
    # x pass-through for resid0: G2 -> even, G3 -> odd, all channels 1
    ones = np.ones((1, 8), np.float32)
    w0x[16:24, 128:192] = np.kron(eye8, ones)
    w0x[24:32, 192:256] = np.kron(eye8, ones)
    # pack [w0x | kron(eye16, U_0)] into one early-DMA tensor
    w0r = np.zeros((128, 384), dtype=np.float32)
    w0r[0:32, 0:256] = w0x
    w0r[:, 256:384] = np.kron(eye16, io_kernels[0, 0])

    # conv taps layers 1..9: [128, 27*128], kron(eye16, W_t)
    wt = np.zeros((128, 27 * 128), dtype=np.float32)
    for i in range(9):
        for t in range(3):
            wt[:, ((i * 3) + t) * 128:((i * 3) + t + 1) * 128] = np.kron(
                eye16, c_kernels[i, t]
            )

    # resid layers 1..8: kron(eye16, U_i)
    wr = np.zeros((128, 8 * 128), dtype=np.float32)
    for i in range(1, 9):
        wr[:, (i - 1) * 128:i * 128] = np.kron(eye16, io_kernels[i, 0])

    # mixer: per layer [128,16]: both parities block-diag
    wm = np.zeros((128, 400), dtype=np.float32)
    for i in range(N_LAYERS):
        blk = np.kron(eye8, mixer_kernel[0, i * 8:(i + 1) * 8, 0][:, None])
        wm[0:64, i * 40:i * 40 + 8] = blk
        wm[64:128, i * 40 + 32:i * 40 + 40] = blk

    # conv biases with io biases folded through the conv taps
    cb = np.zeros((8, N_LAYERS), dtype=np.float64)
    kappa = np.zeros(8, dtype=np.float64)
    for i in range(N_LAYERS):
        if i == 0:
            adj = np.zeros(8)
        else:
            adj = np.einsum("kio,i->o", c_kernels[i - 1].astype(np.float64),
                            kappa)
        cb[:, i] = c_biases[i].astype(np.float64) + adj
        if i < N_LAYERS - 1:
            kappa = kappa + io_biases[i].astype(np.float64)
    cb = np.tile(cb.astype(np.float32), (16, 1))  # [128, 10]
    cbmb = np.zeros((128, 11), np.float32)
    cbmb[:, 0:10] = cb
    cbmb[0:40, 10] = float(np.asarray(mixer_bias).reshape(-1)[0])
    return dict(
        w0r=np.ascontiguousarray(w0r.astype(NPBF16)),
        wt=np.ascontiguousarray(wt.astype(NPBF16)),
        wr=np.ascontiguousarray(wr.astype(NPBF16)),
        wm=np.ascontiguousarray(wm.astype(NPBF16)),
        cbmb=cbmb,
    )


_NC_CACHE = None


def _get_nc():
    global _NC_CACHE
    if _NC_CACHE is None:
        _NC_CACHE = _build_program()
    return _NC_CACHE


def run(inputs, trace=False, **spmd_kwargs):
    """Run on 8 cores; returns (full_output [64,128], BassKernelResults)."""
    x = np.asarray(inputs["x"], dtype=np.float32)
    shared = _host_weights(
        np.asarray(inputs["c0_kernel"], np.float32),
        np.asarray(inputs["c_kernels"], np.float32),
        np.asarray(inputs["c_biases"], np.float32),
        np.asarray(inputs["io_kernels"], np.float32),
        np.asarray(inputs["io_biases"], np.float32),
        np.asarray(inputs["mixer_kernel"], np.float32),
        np.asarray(inputs["mixer_bias"], np.float32),
    )
    xw = x[:, T - W_X:]  # [64, 2174]
    in_maps = []
    for c in range(N_CORES):
        xc = xw[c * B_LOC:(c + 1) * B_LOC]  # [8, 2174]
        xg = np.zeros((32, XGW), dtype=np.float32)
        for g in range(4):
            # G_g[b, j] = x[b, 2j + g], j < HW_Y[0]
            sl = xc[:, g:g + 2 * HW_Y[0]:2]
            xg[g * 8:(g + 1) * 8, :sl.shape[1]] = sl
        # x broadcast for the layer-0 residual: XB[par*64+b*8+ch, j] =
        # x-passthrough value = XG[16 + par*8 + b, j] for all ch
        xb = np.zeros((128, XGW), dtype=np.float32)
        for par in range(2):
            for b in range(8):
                xb[par * 64 + b * 8:par * 64 + b * 8 + 8, :] = (
                    xg[16 + par * 8 + b, :][None, :]
                )
        m = dict(shared)
        m["xg"] = np.ascontiguousarray(xg.astype(NPBF16))
        m["xb"] = np.ascontiguousarray(xb.astype(NPBF16))
        in_maps.append(m)
    nc = _get_nc()
    res = run_bass_kernel_spmd(
        nc, in_maps, core_ids=list(range(N_CORES)), trace=trace, **spmd_kwargs
    )
    out = np.concatenate([res.results[c]["out"] for c in range(N_CORES)], axis=0)
    return out.astype(np.float32), res


def kernel(**inputs):
    out, _ = run(inputs, trace=False)
    return out
